# revision 11
# baseline (speedup 1.0000x reference)
"""Trainium2 Bass kernel for nn_CustomMatMul (BFP block-quantized batched GEMM).

Reference computation (per head h):
    rhs_t = rhs[h].T                      # [N, K]
    lhs_q = bfp_quant(lhs[h])             # blocks of 32 along K, 8-bit mantissa
    rhs_q = bfp_quant(rhs_t)
    out[h] = lhs_q @ rhs_q.T              # [M, N]

Shapes: lhs [1, 16, 4096, 64], rhs [1, 16, 64, 4096] -> out [1, 16, 4096, 4096].

Sharding: 16 heads data-parallel over 8 cores (2 heads/core), no communication.

Device strategy (per core):
  * BFP quantization is exact integer/bit math on the vector engine:
      maxabs per 32-block (abs-max reduce), exponent via bit shift of the f32
      representation (floor(log2(max)) == exponent field, exactly), step and
      1/step built by exponent-field arithmetic (powers of two, exact),
      round-half-even via the +-1.5*2^23 magic-add trick, clip via min/max.
  * Quantized values (8-bit signed mantissa x power-of-2 step) are EXACTLY
    representable in bf16, so the GEMM runs on the tensor engine in bf16:
    products are exact, accumulation is f32 in PSUM - numerically identical
    to the f32 reference up to summation order.
  * lhs is DMA'd in a contiguous layout (partition p holds rows 32p..32p+31),
    quantized in [M, K] layout, then PE-transposed (bf16) into [K, M]
    stationary tiles whose column order is the row-interleaved permutation;
    output stores undo the permutation with a strided partition base.
  * rhs arrives as [K, N]; 128-column chunks are PE-transposed to [N, K]
    (f32), quantized, and PE-transposed back (bf16).
  * Output tiles [128, 512] f32 accumulate in PSUM, are evacuated by the
    scalar/vector engines into a [128, 4096] SBUF staging tile, and leave by
    2 MiB DMA stores. The kernel is bound by the 128 MiB/core output write.
  * Head h+1's quantization work is emitted interleaved into head h's matmul
    phase so the vector engine never stalls PSUM evacuation (and the DMA
    stream never starves) while quantizing the next head.
"""

import sys

sys.path.insert(0, "/opt/trn_rl_repo")

import numpy as np

import concourse.bacc as bacc
from concourse import mybir
from concourse.tile import TileContext
from concourse.bass_utils import run_bass_kernel_spmd

F32 = mybir.dt.float32
BF16 = mybir.dt.bfloat16
I32 = mybir.dt.int32
ALU = mybir.AluOpType
AXL = mybir.AxisListType

N_CORES = 8
H_TOTAL = 16
H_PER_CORE = H_TOTAL // N_CORES
M_FULL = 4096
N_FULL = 4096
K = 64
BLOCK = 32

MAGIC = 12582912.0  # 1.5 * 2^23: adding it rounds to integer (half-to-even)
CLIP_HI = MAGIC + 127.0
CLIP_LO = MAGIC - 128.0
# f32 max clamp for block maxima: 2^-63 guards all-zero blocks (quantizes to 0)
MAXABS_FLOOR = 1.0842021724855044e-19


def _emit_bfp_quant(nc, pools, x_ap, out_ap, ngrp):
    """Quantize x_ap [128, ngrp, 64] f32 -> out_ap [128, ngrp, 64] (bf16).

    Blocks of 32 along the last axis. x_ap may live in SBUF or PSUM.
    """
    qtmp, ytmp = pools
    nb = ngrp * 2
    x4 = x_ap.rearrange("p t (b w) -> p t b w", w=BLOCK)

    mx = qtmp.tile([128, nb], F32, name="q_mx", tag="q_mx")
    nc.vector.tensor_reduce(
        out=mx, in_=x4, axis=AXL.X, op=ALU.max, apply_absolute_value=True
    )
    nc.vector.tensor_scalar(
        out=mx, in0=mx, scalar1=MAXABS_FLOOR, scalar2=None, op0=ALU.max
    )
    eb = qtmp.tile([128, nb], I32, name="q_eb", tag="q_eb")
    nc.vector.tensor_scalar(
        out=eb, in0=mx.bitcast(I32), scalar1=23, scalar2=None,
        op0=ALU.logical_shift_right,
    )
    # inv_e = 261 - eb  (biased exponent of 2^(7-e));  step_e = eb - 7
    t1 = qtmp.tile([128, nb], I32, name="q_t1", tag="q_t1")
    nc.vector.tensor_scalar(
        out=t1, in0=eb, scalar1=-1, scalar2=261, op0=ALU.mult, op1=ALU.add
    )
    inv = qtmp.tile([128, nb], F32, name="q_inv", tag="q_inv")
    nc.vector.tensor_scalar(
        out=inv.bitcast(I32), in0=t1, scalar1=23, scalar2=None,
        op0=ALU.logical_shift_left,
    )
    t2 = qtmp.tile([128, nb], I32, name="q_t2", tag="q_t2")
    nc.vector.tensor_scalar(out=t2, in0=eb, scalar1=-7, scalar2=None, op0=ALU.add)
    stp = qtmp.tile([128, nb], F32, name="q_stp", tag="q_stp")
    nc.vector.tensor_scalar(
        out=stp.bitcast(I32), in0=t2, scalar1=23, scalar2=None,
        op0=ALU.logical_shift_left,
    )

    inv_b = inv.rearrange("p (t b) -> p t b", b=2).unsqueeze(3).broadcast_to(
        [128, ngrp, 2, BLOCK]
    )
    stp_b = stp.rearrange("p (t b) -> p t b", b=2).unsqueeze(3).broadcast_to(
        [128, ngrp, 2, BLOCK]
    )

    y = ytmp.tile([128, ngrp, 64], F32, name="q_y", tag="q_y")
    y4 = y.rearrange("p t (b w) -> p t b w", w=BLOCK)
    yf = y.rearrange("p t k -> p (t k)")
    nc.vector.tensor_tensor(out=y4, in0=x4, in1=inv_b, op=ALU.mult)
    nc.vector.tensor_scalar(
        out=yf, in0=yf, scalar1=MAGIC, scalar2=CLIP_HI, op0=ALU.add, op1=ALU.min
    )
    nc.vector.tensor_scalar(
        out=yf, in0=yf, scalar1=CLIP_LO, scalar2=MAGIC, op0=ALU.max, op1=ALU.subtract
    )
    out4 = out_ap.rearrange("p t (b w) -> p t b w", w=BLOCK)
    nc.vector.tensor_tensor(out=out4, in0=y4, in1=stp_b, op=ALU.mult)


def build_nc(n_heads=H_PER_CORE, m=M_FULL, n=N_FULL):
    nc = bacc.Bacc("TRN2", target_bir_lowering=False, debug=False)
    lhs_d = nc.dram_tensor("lhs", [n_heads, m, K], F32, kind="ExternalInput")
    rhs_d = nc.dram_tensor("rhs", [n_heads, K, n], F32, kind="ExternalInput")
    out_d = nc.dram_tensor("out", [n_heads, m, n], F32, kind="ExternalOutput")

    import ml_dtypes
    id_bf_d = nc.inline_tensor(np.eye(128, dtype=ml_dtypes.bfloat16),
                               name="id_bf_const")
    id_f32_d = nc.inline_tensor(np.eye(64, dtype=np.float32), name="id_f32_const")

    mt = m // 128   # M-tiles per head (row-interleaved blocks)
    nch = n // 128  # 128-wide N-chunks per head
    nq = n // 512   # 512-wide output chunks per M-tile
    GQ = min(8, mt, nch)  # tiles per quantization group (1 PSUM bank)
    assert mt % GQ == 0 and nch % GQ == 0
    ngrp = mt // GQ  # quant groups per head per operand

    with TileContext(nc) as tc:
        with (
            tc.tile_pool(name="persist", bufs=1) as persist,
            tc.tile_pool(name="qtmp", bufs=2) as qtmp,
            tc.tile_pool(name="ytmp", bufs=2) as ytmp,
            tc.tile_pool(name="rtq", bufs=2) as rtq_pool,
            tc.tile_pool(name="stage", bufs=4) as stage_pool,
            tc.tile_pool(name="ps_mm", bufs=4, space="PSUM") as ps_mm,
            tc.tile_pool(name="ps_rt", bufs=2, space="PSUM") as ps_rt,
            tc.tile_pool(name="ps_tq", bufs=2, space="PSUM") as ps_tq,
        ):
            id_bf = persist.tile([128, 128], BF16, name="id_bf", tag="id_bf")
            nc.sync.dma_start(out=id_bf, in_=id_bf_d[:])
            id_f32 = persist.tile([64, 64], F32, name="id_f32", tag="id_f32")
            nc.sync.dma_start(out=id_f32, in_=id_f32_d[:])

            lhs_sb = persist.tile([128, n_heads * mt, K], F32, name="lhs_sb",
                                  tag="lhs_sb")
            rhs_sb = persist.tile([64, n_heads, n], F32, name="rhs_sb", tag="rhs_sb")
            lhs_q = persist.tile([128, n_heads * mt, K], BF16, name="lhs_q",
                                 tag="lhs_q")
            lhsT_q = persist.tile([64, n_heads * mt, 128], BF16, name="lhsT_q",
                                  tag="lhsT_q")
            rhs_q = persist.tile([64, n_heads * nch, 128], BF16, name="rhs_q",
                                 tag="rhs_q")

            # ---- input loads (issued up front, overlap everything) ----
            # lhs: contiguous per-partition layout: partition p holds rows
            # [wpp*p, wpp*(p+1)); free index w picks the row within the block.
            wpp = mt  # rows per partition = m / 128 = mt
            for h in range(n_heads):
                nc.sync.dma_start(out=rhs_sb[:, h, :], in_=rhs_d[h][:])
                nc.sync.dma_start(
                    out=lhs_sb[:, h * mt:(h + 1) * mt, :],
                    in_=lhs_d[h].rearrange("(p w) k -> p w k", w=wpp),
                )

            def emit_quant_job(h, kind, g):
                if kind == "r":
                    # rhs: PE-transpose f32 chunks -> quantize -> transpose back
                    base = h * nch + g * GQ
                    prt = ps_rt.tile([128, GQ, K], F32, name="prt", tag="ps_rt")
                    for j in range(GQ):
                        c = g * GQ + j
                        nc.tensor.transpose(
                            out=prt[:, j, :],
                            in_=rhs_sb[:, h, c * 128:(c + 1) * 128],
                            identity=id_f32,
                        )
                    rtq = rtq_pool.tile([128, GQ, K], BF16, name="rtq", tag="rtq")
                    _emit_bfp_quant(nc, (qtmp, ytmp), prt, rtq, GQ)
                    pt = ps_tq.tile([64, GQ, 128], BF16, name="pt_r", tag="ps_tq")
                    for j in range(GQ):
                        nc.tensor.transpose(
                            out=pt[:, j, :], in_=rtq[:, j, :], identity=id_bf
                        )
                    nc.scalar.copy(out=rhs_q[:, base:base + GQ, :], in_=pt)
                else:
                    # lhs: quantize in [M, K] layout, then PE-transpose to [K, M]
                    base = h * mt + g * GQ
                    _emit_bfp_quant(
                        nc, (qtmp, ytmp),
                        lhs_sb[:, base:base + GQ, :],
                        lhs_q[:, base:base + GQ, :], GQ,
                    )
                    pt = ps_tq.tile([64, GQ, 128], BF16, name="pt_l", tag="ps_tq")
                    for j in range(GQ):
                        nc.tensor.transpose(
                            out=pt[:, j, :], in_=lhs_q[:, base + j, :], identity=id_bf
                        )
                    nc.scalar.copy(out=lhsT_q[:, base:base + GQ, :], in_=pt)

            def quant_jobs(h):
                """rhs/lhs groups interleaved so early matmuls unblock fast."""
                jobs = []
                ngrp_r, ngrp_l = nch // GQ, mt // GQ
                for g in range(max(ngrp_r, ngrp_l)):
                    if g < ngrp_r:
                        jobs.append(("r", g))
                    if g < ngrp_l:
                        jobs.append(("l", g))
                return jobs

            # head 0 quantization up front
            for kind, g in quant_jobs(0):
                emit_quant_job(0, kind, g)

            # ---- matmuls + evacuation + stores, with next head's quant
            # interleaved ----
            # out rows of M-tile r (head h): {wpp*j + r : j in 0..127}
            out_v = out_d.rearrange("h (j w) n -> h w j n", w=wpp)
            for h in range(n_heads):
                next_jobs = list(quant_jobs(h + 1)) if h + 1 < n_heads else []
                interleave_every = max(1, (mt - 4) // max(1, len(next_jobs)))
                for t in range(mt):
                    stage = stage_pool.tile([128, n], F32, name="stage", tag="stage")
                    for j in range(nq):
                        ps = ps_mm.tile([128, 512], F32, name="ps_o", tag="ps_mm")
                        nc.tensor.matmul(
                            out=ps,
                            lhsT=lhsT_q[:, h * mt + t, :],
                            rhs=rhs_q[:, h * nch + 4 * j:h * nch + 4 * j + 4, :],
                            start=True, stop=True,
                        )
                        dst = stage[:, j * 512:(j + 1) * 512]
                        if j % 2 == 0:
                            nc.scalar.copy(out=dst, in_=ps)
                        else:
                            nc.vector.tensor_copy(out=dst, in_=ps)
                    nc.sync.dma_start(out=out_v[h, t], in_=stage)
                    if next_jobs and t >= 1 and (t - 1) % interleave_every == 0:
                        kind, grp = next_jobs.pop(0)
                        emit_quant_job(h + 1, kind, grp)
                # small configs: flush any quant jobs that didn't fit
                for kind, grp in next_jobs:
                    emit_quant_job(h + 1, kind, grp)
    nc.finalize()
    return nc


_NC_CACHE = {}


def _get_nc(n_heads=H_PER_CORE, m=M_FULL, n=N_FULL):
    key = (n_heads, m, n)
    if key not in _NC_CACHE:
        _NC_CACHE[key] = build_nc(n_heads, m, n)
    return _NC_CACHE[key]


def run_sharded(lhs_mat, rhs_mat, trace=False, **kwargs):
    """Shard over 8 cores, run, gather. Returns (out, BassKernelResults)."""
    lhs = np.asarray(lhs_mat).reshape(H_TOTAL, M_FULL, K)
    rhs = np.asarray(rhs_mat).reshape(H_TOTAL, K, N_FULL)
    in_maps = []
    for c in range(N_CORES):
        h0 = c * H_PER_CORE
        in_maps.append(
            {
                "lhs": np.ascontiguousarray(lhs[h0:h0 + H_PER_CORE]),
                "rhs": np.ascontiguousarray(rhs[h0:h0 + H_PER_CORE]),
            }
        )
    nc = _get_nc()
    res = run_bass_kernel_spmd(
        nc, in_maps, list(range(N_CORES)), trace=trace, **kwargs
    )
    out = np.concatenate([r["out"] for r in res.results], axis=0)
    return out.reshape(1, H_TOTAL, M_FULL, N_FULL), res


def kernel(lhs_mat, rhs_mat):
    out, _ = run_sharded(lhs_mat, rhs_mat, trace=False)
    return out


# revision 12
# speedup vs baseline: 1.2007x; 1.2007x over previous
"""Trainium2 Bass kernel for nn_CustomMatMul (BFP block-quantized batched GEMM).

Reference computation (per head h):
    rhs_t = rhs[h].T                      # [N, K]
    lhs_q = bfp_quant(lhs[h])             # blocks of 32 along K, 8-bit mantissa
    rhs_q = bfp_quant(rhs_t)
    out[h] = lhs_q @ rhs_q.T              # [M, N]

Shapes: lhs [1, 16, 4096, 64], rhs [1, 16, 64, 4096] -> out [1, 16, 4096, 4096].

Sharding: 16 heads data-parallel over 8 cores (2 heads/core), no communication.

Device strategy (per core):
  * BFP quantization is exact integer/bit math on the vector engine:
      maxabs per 32-block (abs-max reduce), exponent via bit shift of the f32
      representation (floor(log2(max)) == exponent field, exactly), step and
      1/step built by exponent-field arithmetic (powers of two, exact),
      round-half-even via the +-1.5*2^23 magic-add trick, clip via min/max.
  * Quantized values (8-bit signed mantissa x power-of-2 step) are EXACTLY
    representable in bf16, so the GEMM runs on the tensor engine in bf16:
    products are exact, accumulation is f32 in PSUM - numerically identical
    to the f32 reference up to summation order.
  * lhs is DMA'd in a contiguous layout (partition p holds rows 32p..32p+31),
    quantized in [M, K] layout, then PE-transposed (bf16) into [K, M]
    stationary tiles whose column order is the row-interleaved permutation;
    output stores undo the permutation with a strided partition base.
  * rhs arrives as [K, N]; 128-column chunks are PE-transposed to [N, K]
    (f32), quantized, and PE-transposed back (bf16).
  * Output tiles [128, 512] f32 accumulate in PSUM, are evacuated by the
    scalar/vector engines into a [128, 4096] SBUF staging tile, and leave by
    2 MiB DMA stores. The kernel is bound by the 128 MiB/core output write.
  * Head h+1's quantization work is emitted interleaved into head h's matmul
    phase so the vector engine never stalls PSUM evacuation (and the DMA
    stream never starves) while quantizing the next head.
"""

import sys

sys.path.insert(0, "/opt/trn_rl_repo")

import numpy as np

import concourse.bacc as bacc
from concourse import mybir
from concourse.tile import TileContext
from concourse.bass_utils import run_bass_kernel_spmd

F32 = mybir.dt.float32
BF16 = mybir.dt.bfloat16
I32 = mybir.dt.int32
ALU = mybir.AluOpType
AXL = mybir.AxisListType

N_CORES = 8
H_TOTAL = 16
H_PER_CORE = H_TOTAL // N_CORES
M_FULL = 4096
N_FULL = 4096
K = 64
BLOCK = 32

MAGIC = 12582912.0  # 1.5 * 2^23: adding it rounds to integer (half-to-even)
CLIP_HI = MAGIC + 127.0
CLIP_LO = MAGIC - 128.0
# f32 max clamp for block maxima: 2^-63 guards all-zero blocks (quantizes to 0)
MAXABS_FLOOR = 1.0842021724855044e-19


def _emit_bfp_quant(nc, pools, x_ap, out_ap, ngrp):
    """Quantize x_ap [128, ngrp, 64] f32 -> out_ap [128, ngrp, 64] (bf16).

    Blocks of 32 along the last axis. x_ap may live in SBUF or PSUM.
    """
    qtmp, ytmp = pools
    nb = ngrp * 2
    x4 = x_ap.rearrange("p t (b w) -> p t b w", w=BLOCK)

    mx = qtmp.tile([128, nb], F32, name="q_mx", tag="q_mx")
    nc.vector.tensor_reduce(
        out=mx, in_=x4, axis=AXL.X, op=ALU.max, apply_absolute_value=True
    )
    nc.vector.tensor_scalar(
        out=mx, in0=mx, scalar1=MAXABS_FLOOR, scalar2=None, op0=ALU.max
    )
    eb = qtmp.tile([128, nb], I32, name="q_eb", tag="q_eb")
    nc.vector.tensor_scalar(
        out=eb, in0=mx.bitcast(I32), scalar1=23, scalar2=None,
        op0=ALU.logical_shift_right,
    )
    # inv_e = 261 - eb  (biased exponent of 2^(7-e));  step_e = eb - 7
    t1 = qtmp.tile([128, nb], I32, name="q_t1", tag="q_t1")
    nc.vector.tensor_scalar(
        out=t1, in0=eb, scalar1=-1, scalar2=261, op0=ALU.mult, op1=ALU.add
    )
    inv = qtmp.tile([128, nb], F32, name="q_inv", tag="q_inv")
    nc.vector.tensor_scalar(
        out=inv.bitcast(I32), in0=t1, scalar1=23, scalar2=None,
        op0=ALU.logical_shift_left,
    )
    t2 = qtmp.tile([128, nb], I32, name="q_t2", tag="q_t2")
    nc.vector.tensor_scalar(out=t2, in0=eb, scalar1=-7, scalar2=None, op0=ALU.add)
    stp = qtmp.tile([128, nb], F32, name="q_stp", tag="q_stp")
    nc.vector.tensor_scalar(
        out=stp.bitcast(I32), in0=t2, scalar1=23, scalar2=None,
        op0=ALU.logical_shift_left,
    )

    inv_b = inv.rearrange("p (t b) -> p t b", b=2).unsqueeze(3).broadcast_to(
        [128, ngrp, 2, BLOCK]
    )
    stp_b = stp.rearrange("p (t b) -> p t b", b=2).unsqueeze(3).broadcast_to(
        [128, ngrp, 2, BLOCK]
    )

    y = ytmp.tile([128, ngrp, 64], F32, name="q_y", tag="q_y")
    y4 = y.rearrange("p t (b w) -> p t b w", w=BLOCK)
    yf = y.rearrange("p t k -> p (t k)")
    nc.vector.tensor_tensor(out=y4, in0=x4, in1=inv_b, op=ALU.mult)
    nc.vector.tensor_scalar(
        out=yf, in0=yf, scalar1=MAGIC, scalar2=CLIP_HI, op0=ALU.add, op1=ALU.min
    )
    nc.vector.tensor_scalar(
        out=yf, in0=yf, scalar1=CLIP_LO, scalar2=MAGIC, op0=ALU.max, op1=ALU.subtract
    )
    out4 = out_ap.rearrange("p t (b w) -> p t b w", w=BLOCK)
    nc.vector.tensor_tensor(out=out4, in0=y4, in1=stp_b, op=ALU.mult)


def build_nc(n_heads=H_PER_CORE, m=M_FULL, n=N_FULL):
    nc = bacc.Bacc("TRN2", target_bir_lowering=False, debug=False)
    lhs_d = nc.dram_tensor("lhs", [n_heads, m, K], F32, kind="ExternalInput")
    rhs_d = nc.dram_tensor("rhs", [n_heads, K, n], F32, kind="ExternalInput")
    out_d = nc.dram_tensor("out", [n_heads, m, n], F32, kind="ExternalOutput")

    import ml_dtypes
    id_bf_d = nc.inline_tensor(np.eye(128, dtype=ml_dtypes.bfloat16),
                               name="id_bf_const")

    id_f32_d = nc.inline_tensor(
        np.tile(np.eye(64, dtype=np.float32), (n_heads, 1)), name="id_f32_const")

    mt = m // 128   # M-tiles per head (row-interleaved blocks)
    nch = n // 128  # 128-wide N-chunks per head
    nq = n // 512   # 512-wide output chunks per M-tile
    GQ = min(8, mt, nch)  # tiles per quantization group (1 PSUM bank)
    assert mt % GQ == 0 and nch % GQ == 0
    ngrp = mt // GQ  # quant groups per head per operand

    with TileContext(nc) as tc:
        with (
            tc.tile_pool(name="persist", bufs=1) as persist,
            tc.tile_pool(name="qtmp", bufs=2) as qtmp,
            tc.tile_pool(name="ytmp", bufs=2) as ytmp,
            tc.tile_pool(name="rtq", bufs=2) as rtq_pool,
            tc.tile_pool(name="stage", bufs=4) as stage_pool,
            tc.tile_pool(name="ps_mm", bufs=4, space="PSUM") as ps_mm,
            tc.tile_pool(name="ps_rt", bufs=2, space="PSUM") as ps_rt,
            tc.tile_pool(name="ps_tq", bufs=2, space="PSUM") as ps_tq,
        ):
            id_bf = persist.tile([128, 128], BF16, name="id_bf", tag="id_bf")
            nc.sync.dma_start(out=id_bf, in_=id_bf_d[:])
            id_f32 = persist.tile([64 * n_heads, 64], F32, name="id_f32",
                                  tag="id_f32")
            nc.sync.dma_start(out=id_f32, in_=id_f32_d[:])

            lhs_sb = persist.tile([128, n_heads * mt, K], F32, name="lhs_sb",
                                  tag="lhs_sb")
            rhs_sb = persist.tile([64 * n_heads, n], F32, name="rhs_sb",
                                  tag="rhs_sb")
            lhs_q = persist.tile([128, n_heads * mt, K], BF16, name="lhs_q",
                                 tag="lhs_q")
            lhsT_q = persist.tile([64, n_heads * mt, 128], BF16, name="lhsT_q",
                                  tag="lhsT_q")
            rhs_q = persist.tile([64, n_heads * nch, 128], BF16, name="rhs_q",
                                 tag="rhs_q")

            # ---- input loads (issued up front, overlap everything) ----
            # lhs: contiguous per-partition layout: partition p holds rows
            # [wpp*p, wpp*(p+1)); free index w picks the row within the block.
            wpp = mt  # rows per partition = m / 128 = mt
            rhs_flat = rhs_d.rearrange("h k n -> (h k) n")
            gw = n // max(1, nch // GQ)  # columns per rhs quant group
            nc.sync.dma_start(out=rhs_sb[:, 0:gw], in_=rhs_flat[:, 0:gw])
            nc.sync.dma_start(
                out=lhs_sb[:, 0:mt, :],
                in_=lhs_d[0].rearrange("(p w) k -> p w k", w=wpp),
            )
            for g in range(1, nch // GQ):
                nc.sync.dma_start(
                    out=rhs_sb[:, g * gw:(g + 1) * gw],
                    in_=rhs_flat[:, g * gw:(g + 1) * gw],
                )
            for h in range(1, n_heads):
                nc.sync.dma_start(
                    out=lhs_sb[:, h * mt:(h + 1) * mt, :],
                    in_=lhs_d[h].rearrange("(p w) k -> p w k", w=wpp),
                )

            def emit_quant_job(h, kind, g):
                if kind == "r":
                    # rhs: PE-transpose f32 chunks -> quantize -> transpose back
                    base = h * nch + g * GQ
                    prt = ps_rt.tile([128, GQ, K], F32, name="prt", tag="ps_rt")
                    for j in range(GQ):
                        c = g * GQ + j
                        nc.tensor.transpose(
                            out=prt[:, j, :],
                            in_=rhs_sb[h * 64:(h + 1) * 64,
                                       c * 128:(c + 1) * 128],
                            identity=id_f32[h * 64:(h + 1) * 64, :],
                        )
                    rtq = rtq_pool.tile([128, GQ, K], BF16, name="rtq", tag="rtq")
                    _emit_bfp_quant(nc, (qtmp, ytmp), prt, rtq, GQ)
                    pt = ps_tq.tile([64, GQ, 128], BF16, name="pt_r", tag="ps_tq")
                    for j in range(GQ):
                        nc.tensor.transpose(
                            out=pt[:, j, :], in_=rtq[:, j, :], identity=id_bf
                        )
                    nc.scalar.copy(out=rhs_q[:, base:base + GQ, :], in_=pt)
                else:
                    # lhs: quantize in [M, K] layout, then PE-transpose to [K, M]
                    base = h * mt + g * GQ
                    _emit_bfp_quant(
                        nc, (qtmp, ytmp),
                        lhs_sb[:, base:base + GQ, :],
                        lhs_q[:, base:base + GQ, :], GQ,
                    )
                    pt = ps_tq.tile([64, GQ, 128], BF16, name="pt_l", tag="ps_tq")
                    for j in range(GQ):
                        nc.tensor.transpose(
                            out=pt[:, j, :], in_=lhs_q[:, base + j, :], identity=id_bf
                        )
                    nc.scalar.copy(out=lhsT_q[:, base:base + GQ, :], in_=pt)

            def quant_jobs(h):
                """rhs/lhs groups interleaved so early matmuls unblock fast."""
                jobs = []
                ngrp_r, ngrp_l = nch // GQ, mt // GQ
                for g in range(max(ngrp_r, ngrp_l)):
                    if g < ngrp_r:
                        jobs.append(("r", g))
                    if g < ngrp_l:
                        jobs.append(("l", g))
                return jobs

            # head 0 quantization up front
            for kind, g in quant_jobs(0):
                emit_quant_job(0, kind, g)

            # ---- matmuls + evacuation + stores, with next head's quant
            # interleaved ----
            # out rows of M-tile r (head h): {wpp*j + r : j in 0..127}
            out_v = out_d.rearrange("h (j w) n -> h w j n", w=wpp)
            for h in range(n_heads):
                next_jobs = list(quant_jobs(h + 1)) if h + 1 < n_heads else []
                interleave_every = max(1, (mt - 4) // max(1, len(next_jobs)))
                for t in range(mt):
                    stage = stage_pool.tile([128, n], F32, name="stage", tag="stage")
                    for j in range(nq):
                        ps = ps_mm.tile([128, 512], F32, name="ps_o", tag="ps_mm")
                        nc.tensor.matmul(
                            out=ps,
                            lhsT=lhsT_q[:, h * mt + t, :],
                            rhs=rhs_q[:, h * nch + 4 * j:h * nch + 4 * j + 4, :],
                            start=True, stop=True,
                        )
                        dst = stage[:, j * 512:(j + 1) * 512]
                        if j % 8 in (1, 3, 5):
                            nc.vector.tensor_copy(out=dst, in_=ps)
                        else:
                            nc.scalar.copy(out=dst, in_=ps)
                    nc.sync.dma_start(out=out_v[h, t], in_=stage)
                    if next_jobs and t >= 1 and (t - 1) % interleave_every == 0:
                        kind, grp = next_jobs.pop(0)
                        emit_quant_job(h + 1, kind, grp)
                # small configs: flush any quant jobs that didn't fit
                for kind, grp in next_jobs:
                    emit_quant_job(h + 1, kind, grp)
    nc.finalize()
    return nc


_NC_CACHE = {}


def _get_nc(n_heads=H_PER_CORE, m=M_FULL, n=N_FULL):
    key = (n_heads, m, n)
    if key not in _NC_CACHE:
        _NC_CACHE[key] = build_nc(n_heads, m, n)
    return _NC_CACHE[key]


def run_sharded(lhs_mat, rhs_mat, trace=False, **kwargs):
    """Shard over 8 cores, run, gather. Returns (out, BassKernelResults)."""
    lhs = np.asarray(lhs_mat).reshape(H_TOTAL, M_FULL, K)
    rhs = np.asarray(rhs_mat).reshape(H_TOTAL, K, N_FULL)
    in_maps = []
    for c in range(N_CORES):
        h0 = c * H_PER_CORE
        in_maps.append(
            {
                "lhs": np.ascontiguousarray(lhs[h0:h0 + H_PER_CORE]),
                "rhs": np.ascontiguousarray(rhs[h0:h0 + H_PER_CORE]),
            }
        )
    nc = _get_nc()
    res = run_bass_kernel_spmd(
        nc, in_maps, list(range(N_CORES)), trace=trace, **kwargs
    )
    out = np.concatenate([r["out"] for r in res.results], axis=0)
    return out.reshape(1, H_TOTAL, M_FULL, N_FULL), res


def kernel(lhs_mat, rhs_mat):
    out, _ = run_sharded(lhs_mat, rhs_mat, trace=False)
    return out
